# revision 1
# baseline (speedup 1.0000x reference)
"""Trainium2 Bass kernel for nn_ComposedFeatureTransformer (NNUE-style double
feature transformer: sparse gather-accumulate + bias, perspective concat, clip,
psqt head).

Strategy: data-parallel over batch across 8 NeuronCores (512 items/core, table
replicated). Per core, items are processed in 4 tiles of 128 (one item per SBUF
partition); each tile needs two 32-row weighted sums (w/b perspectives).

Rows are fetched with indirect DMA (one [128, 3080] row-gather per feature
slot) into a ring of staging tiles; DVE accumulates staging tiles into the
per-perspective accumulator (first add seeds with the broadcast bias), then
combines perspectives (us/them mix + clip + psqt) into an output tile that SP
(HWDGE) streams to DRAM. All DMA writes are plain copies — no in-DMA
accumulate (the SDMA CCE read-modify-write is unsound for multi-packet rows).
The kernel is memory-bound on gather traffic: 512*64 rows * 12320 B ~= 404 MB
per core.

Requires w_values/b_values == 1 (guaranteed by the problem spec fill); falls
back to a host computation otherwise so kernel() stays correct on any input.
"""
import sys

if '/opt/trn_rl_repo' not in sys.path:
    sys.path.insert(0, '/opt/trn_rl_repo')

import numpy as np

import concourse.bass as bass
import concourse.mybir as mybir
from concourse.bass_utils import run_bass_kernel_spmd

L1 = 3072
NPSQT = 8
D = L1 + NPSQT            # 3080
V = 45056                 # table rows
K = 32                    # active features per perspective
B = 4096                  # batch
NCORES = 8
BPC = B // NCORES         # 512 items per core
NT = BPC // 128           # 4 item-tiles per core
OUTD = 2 * L1 + NPSQT     # 6152
# PAIR=2 (two rows per indirect DMA) was measured cost-model-neutral —
# descriptor generation is fully hidden behind the transfer stream — so the
# hardware-validated single-row configuration is kept.
PAIR = 1                  # table rows fetched per indirect DMA
S = 6                     # staging ring depth (slots of PAIR rows)

f32 = mybir.dt.float32
i32 = mybir.dt.int32

_CACHE = {}


def build_nc(ft_max: float, repeat: int = 1):
    """Build the single-core Bass program (shared SPMD across all 8 cores).

    repeat>1 re-runs the whole compute that many times (for timing the
    on-device portion by slope; the output is simply rewritten).
    """
    nc = bass.Bass()
    table = nc.declare_dram_parameter("table", [V, D], f32, isOutput=False)
    biasb = nc.declare_dram_parameter("biasb", [128, D], f32, isOutput=False)
    idxs = nc.declare_dram_parameter("idxs", [128, 2 * NT * K], i32, isOutput=False)
    usth = nc.declare_dram_parameter("usth", [128, 3 * NT], f32, isOutput=False)
    out = nc.declare_dram_parameter("out", [BPC, OUTD], f32, isOutput=True)

    NG = repeat * 2 * NT * K      # total gathers
    LOADS = 48                    # idx + usth + bias preload sem ticks

    with (
        nc.sbuf_tensor([128, 2 * NT * K], i32) as idx_s,
        nc.sbuf_tensor([128, 3 * NT], f32) as usth_s,
        nc.sbuf_tensor([128, D], f32) as bias_s,
        nc.sbuf_tensor([128, D], f32) as acc_w,
        nc.sbuf_tensor([128, D], f32) as acc_b,
        nc.sbuf_tensor([128, S * PAIR * D], f32) as stage,
        nc.sbuf_tensor([128, OUTD], f32) as out_t0,
        nc.sbuf_tensor([128, OUTD], f32) as out_t1,
        nc.sbuf_tensor([128, L1], f32) as tmp,
        nc.semaphore("load_sem") as load_sem,
        nc.semaphore("gsem0") as gsem0,
        nc.semaphore("gsem1") as gsem1,
        nc.semaphore("gsem2") as gsem2,
        nc.semaphore("gsem3") as gsem3,
        nc.semaphore("gsem4") as gsem4,
        nc.semaphore("gsem5") as gsem5,
        nc.semaphore("dve_sem") as dve_sem,
        nc.semaphore("combine_sem") as combine_sem,
        nc.semaphore("osem0") as osem0,
        nc.semaphore("osem1") as osem1,
        nc.Block() as block,
    ):
        # one completion sem per staging slot / output-tile parity: a waiter's
        # threshold can only be satisfied by that slot's own DMA chain, so a
        # fast SDMA engine running ahead on *other* DMAs can't mask a slow
        # engine's unfinished descriptors (cumulative counts on one shared sem
        # are unsound across concurrently in-flight DMAs)
        gsem = [gsem0, gsem1, gsem2, gsem3, gsem4, gsem5][:S]
        assert len(gsem) == S
        osem = [osem0, osem1]
        out_t = [out_t0, out_t1]

        def stg(s, j=0):
            # row j of staging slot s
            return stage[:, (s * PAIR + j) * D:(s * PAIR + j + 1) * D]

        @block.gpsimd
        def _(g):
            g.dma_start(out=idx_s[:], in_=idxs[:]).then_inc(load_sem, 16)
            g.dma_start(out=usth_s[:], in_=usth[:]).then_inc(load_sem, 16)
            g.dma_start(out=bias_s[:], in_=biasb[:]).then_inc(load_sem, 16)
            # descriptor generation reads idx_s from SBUF: loads must land first
            g.wait_ge(load_sem, LOADS)
            assert K % PAIR == 0  # a DMA's rows never straddle units
            for gp in range(NG // PAIR):
                u, k0 = (gp * PAIR) // K, (gp * PAIR) % K
                t, p = (u // 2) % NT, u % 2
                col = (2 * t + p) * K + k0
                if gp >= S:
                    # DVE must have consumed the staging slot's previous tenant
                    g.wait_ge(dve_sem, (gp - S + 1) * PAIR)
                g.indirect_dma_start(
                    out=stage[:, (gp % S) * PAIR * D:((gp % S) + 1) * PAIR * D],
                    out_offset=None,
                    in_=table[:],
                    in_offset=bass.IndirectOffsetOnAxis(
                        ap=idx_s[:, col:col + PAIR], axis=0
                    ),
                ).then_inc(gsem[gp % S], 16)

        @block.vector
        def _(v):
            v.wait_ge(load_sem, LOADS)   # usth_s/bias_s resident
            for it in range(repeat * NT):
                t, pb = it % NT, it % 2
                for p, acc in ((0, acc_w), (1, acc_b)):
                    u = 2 * it + p
                    for k in range(K):
                        gi = u * K + k
                        gp, j = gi // PAIR, gi % PAIR
                        if j == 0:
                            v.wait_ge(gsem[gp % S], 16 * (gp // S + 1))
                        src = bias_s if k == 0 else acc
                        v.tensor_tensor(
                            out=acc[:], in0=src[:], in1=stg(gp % S, j),
                            op=mybir.AluOpType.add,
                        ).then_inc(dve_sem, 1)
                if it >= 2:
                    # SP must have drained out_t[pb] (tile it-2)
                    v.wait_ge(osem[pb], 16 * ((it - 2) // 2 + 1))
                w, b, o = acc_w, acc_b, out_t[pb]
                us = usth_s[:, t:t + 1]
                them = usth_s[:, NT + t:NT + t + 1]
                ush = usth_s[:, 2 * NT + t:2 * NT + t + 1]
                # o[:, :L1] = clip(us*w + them*b); o[:, L1:2L1] = clip(us*b + them*w)
                v.tensor_scalar_mul(tmp[:], b[:, :L1], them)
                v.scalar_tensor_tensor(
                    o[:, 0:L1], w[:, :L1], us, tmp[:],
                    op0=mybir.AluOpType.mult, op1=mybir.AluOpType.add,
                )
                v.tensor_scalar(
                    o[:, 0:L1], o[:, 0:L1], 0.0, ft_max,
                    op0=mybir.AluOpType.max, op1=mybir.AluOpType.min,
                )
                v.tensor_scalar_mul(tmp[:], w[:, :L1], them)
                v.scalar_tensor_tensor(
                    o[:, L1:2 * L1], b[:, :L1], us, tmp[:],
                    op0=mybir.AluOpType.mult, op1=mybir.AluOpType.add,
                )
                v.tensor_scalar(
                    o[:, L1:2 * L1], o[:, L1:2 * L1], 0.0, ft_max,
                    op0=mybir.AluOpType.max, op1=mybir.AluOpType.min,
                )
                # psqt = (w_psqt - b_psqt) * (us - 0.5); bias cancels in the diff
                v.tensor_tensor(
                    out=tmp[:, :NPSQT], in0=w[:, L1:D], in1=b[:, L1:D],
                    op=mybir.AluOpType.subtract,
                )
                v.tensor_scalar_mul(
                    o[:, 2 * L1:OUTD], tmp[:, :NPSQT], ush
                ).then_inc(combine_sem, 1)

        @block.sync
        def _(s):
            for it in range(repeat * NT):
                t, pb = it % NT, it % 2
                s.wait_ge(combine_sem, it + 1)
                s.dma_start(
                    out=out[t * 128:(t + 1) * 128, :], in_=out_t[pb][:]
                ).then_inc(osem[pb], 16)
            n = repeat * NT
            s.wait_ge(osem[0], 16 * ((n + 1) // 2))
            s.wait_ge(osem[1], 16 * (n // 2))

    return nc


def _prep_core_inputs(c, table, biasb, w_idx, b_idx, us, them):
    sl = slice(c * BPC, (c + 1) * BPC)
    wi = w_idx[sl].reshape(NT, 128, K)
    bi = b_idx[sl].reshape(NT, 128, K)
    blocks = []
    for t in range(NT):
        blocks.append(wi[t])
        blocks.append(bi[t])
    idxs = np.ascontiguousarray(np.concatenate(blocks, axis=1), dtype=np.int32)
    us_c = np.ascontiguousarray(us[sl, 0].reshape(NT, 128).T, dtype=np.float32)
    th_c = np.ascontiguousarray(them[sl, 0].reshape(NT, 128).T, dtype=np.float32)
    usth = np.concatenate([us_c, th_c, us_c - 0.5], axis=1).astype(np.float32)
    return {"table": table, "biasb": biasb, "idxs": idxs, "usth": usth}


def run_on_hw(w_indices, w_values, b_indices, b_values, us, them, ft_max_val,
              merged_weight, bias, trace=False, repeat=1):
    """Run the device kernel; returns (output [B, OUTD], BassKernelResults)."""
    ft_max = float(np.asarray(ft_max_val))
    key = ("nc", ft_max, repeat)
    if key not in _CACHE:
        _CACHE[key] = build_nc(ft_max, repeat)
    nc = _CACHE[key]

    table = np.ascontiguousarray(merged_weight, dtype=np.float32)
    biasb = np.ascontiguousarray(
        np.broadcast_to(np.asarray(bias, dtype=np.float32), (128, D))
    )
    w_idx = np.asarray(w_indices, dtype=np.int64)
    b_idx = np.asarray(b_indices, dtype=np.int64)
    us = np.asarray(us, dtype=np.float32)
    them = np.asarray(them, dtype=np.float32)

    in_maps = [
        _prep_core_inputs(c, table, biasb, w_idx, b_idx, us, them)
        for c in range(NCORES)
    ]
    res = run_bass_kernel_spmd(nc, in_maps, list(range(NCORES)), trace=trace)
    outp = np.concatenate([res.results[c]["out"] for c in range(NCORES)], axis=0)
    return outp, res


def _host_fallback(w_indices, w_values, b_indices, b_values, us, them,
                   ft_max_val, merged_weight, bias):
    def acc(idx, val):
        rows = merged_weight[idx]
        return np.einsum('bk,bkd->bd', val, rows) + bias
    w = acc(w_indices, w_values)
    b = acc(b_indices, b_values)
    wacc, wpsqt = w[:, :L1], w[:, L1:]
    bacc, bpsqt = b[:, :L1], b[:, L1:]
    l0 = us * np.concatenate([wacc, bacc], axis=1) \
        + them * np.concatenate([bacc, wacc], axis=1)
    l0 = np.clip(l0, 0.0, np.float32(float(np.asarray(ft_max_val))))
    psqt = (wpsqt - bpsqt) * (us - 0.5)
    return np.concatenate([l0, psqt], axis=1).astype(np.float32)


def kernel(w_indices, w_values, b_indices, b_values, us, them, ft_max_val,
           merged_weight, bias):
    if not (np.all(np.asarray(w_values) == 1.0)
            and np.all(np.asarray(b_values) == 1.0)):
        # the device program folds the unit feature values into plain
        # accumulation; anything else is out of spec — stay correct on host
        return _host_fallback(w_indices, w_values, b_indices, b_values, us,
                              them, ft_max_val, merged_weight, bias)
    outp, _ = run_on_hw(w_indices, w_values, b_indices, b_values, us, them,
                        ft_max_val, merged_weight, bias)
    return outp



# revision 16
# speedup vs baseline: 1.2966x; 1.2966x over previous
"""Trainium2 Bass kernel for nn_ComposedFeatureTransformer (NNUE-style double
feature transformer: sparse gather-accumulate + bias, perspective concat, clip,
psqt head).

Strategy: data-parallel over batch across 8 NeuronCores (512 items/core, table
replicated). Per core, items are processed in 4 tiles of 128 (one item per SBUF
partition); each tile needs two 32-row weighted sums (w/b perspectives).

Rows are fetched with indirect DMA (one [128, 3080] row-gather per feature
slot) into a ring of staging tiles; DVE accumulates staging tiles into the
per-perspective accumulator (first add seeds with the broadcast bias), then
combines perspectives (us/them mix + clip + psqt) into an output tile that SP
(HWDGE) streams to DRAM. All DMA writes are plain copies — no in-DMA
accumulate (the SDMA CCE read-modify-write is unsound for multi-packet rows).

The table, staging tiles, bias and accumulators are all fp16 (halves the
dominant gather traffic: 512*64 rows * 6160 B ~= 202 MB per core) so the
accumulate tensor_tensor adds take the DVE 2x_1p fast path (all-2-byte
operands, 2 col/cycle) and hide (~430 us) under the gather stream (~560 us).
Mixed-dtype adds would run 1x and become the bottleneck — measured 890 us
DVE-busy with a bf16-table/fp32-acc variant. fp16's 10 mantissa bits keep
the exact-emulation rel err at 2.2e-3 against the harness 2e-2 gate
(bf16 acc: 1.8e-2, fp8 table: 2.8e-2 — both rejected).

The combine uses them == 1-us (guaranteed by setup_inputs, checked at entry):
diff = w - b once per tile (fp16, 2x), then each output half is one
scalar_tensor_tensor against us / -us plus a clip; psqt reuses diff.

Requires w_values/b_values == 1 (guaranteed by the problem spec fill); falls
back to a host computation otherwise so kernel() stays correct on any input.
"""
import sys

if '/opt/trn_rl_repo' not in sys.path:
    sys.path.insert(0, '/opt/trn_rl_repo')

import numpy as np

import concourse.bass as bass
import concourse.mybir as mybir
from concourse.bass_utils import run_bass_kernel_spmd

L1 = 3072
NPSQT = 8
D = L1 + NPSQT            # 3080
V = 45056                 # table rows
K = 32                    # active features per perspective
B = 4096                  # batch
NCORES = 8
BPC = B // NCORES         # 512 items per core
NT = BPC // 128           # 4 item-tiles per core
OUTD = 2 * L1 + NPSQT     # 6152
# PAIR=2 (two rows per indirect DMA) was measured cost-model-neutral —
# descriptor generation is fully hidden behind the transfer stream — so the
# hardware-validated single-row configuration is kept.
PAIR = 1                  # table rows fetched per indirect DMA
S = 6                     # staging ring depth (slots of PAIR rows)

f32 = mybir.dt.float32
f16 = mybir.dt.float16
i32 = mybir.dt.int32

_CACHE = {}


def build_nc(ft_max: float, repeat: int = 1):
    """Build the single-core Bass program (shared SPMD across all 8 cores).

    repeat>1 re-runs the whole compute that many times (for timing the
    on-device portion by slope; the output is simply rewritten).
    """
    nc = bass.Bass()
    table = nc.declare_dram_parameter("table", [V, D], f16, isOutput=False)
    biasb = nc.declare_dram_parameter("biasb", [128, D], f16, isOutput=False)
    idxs = nc.declare_dram_parameter("idxs", [128, 2 * NT * K], i32, isOutput=False)
    usth = nc.declare_dram_parameter("usth", [128, 3 * NT], f32, isOutput=False)
    out = nc.declare_dram_parameter("out", [BPC, OUTD], f32, isOutput=True)

    NG = repeat * 2 * NT * K      # total gathers
    LOADS = 48                    # idx + usth + bias preload sem ticks

    with (
        nc.sbuf_tensor([128, 2 * NT * K], i32) as idx_s,
        nc.sbuf_tensor([128, 3 * NT], f32) as usth_s,
        nc.sbuf_tensor([128, D], f16) as bias_s,
        nc.sbuf_tensor([128, D], f16) as acc_w,
        nc.sbuf_tensor([128, D], f16) as acc_b,
        nc.sbuf_tensor([128, S * PAIR * D], f16) as stage,
        nc.sbuf_tensor([128, OUTD], f32) as out_t0,
        nc.sbuf_tensor([128, OUTD], f32) as out_t1,
        nc.sbuf_tensor([128, D], f16) as diff,
        nc.semaphore("load_sem") as load_sem,
        nc.semaphore("gsem0") as gsem0,
        nc.semaphore("gsem1") as gsem1,
        nc.semaphore("gsem2") as gsem2,
        nc.semaphore("gsem3") as gsem3,
        nc.semaphore("gsem4") as gsem4,
        nc.semaphore("gsem5") as gsem5,
        nc.semaphore("dve_sem") as dve_sem,
        nc.semaphore("combine_sem") as combine_sem,
        nc.semaphore("osem0") as osem0,
        nc.semaphore("osem1") as osem1,
        nc.Block() as block,
    ):
        # one completion sem per staging slot / output-tile parity: a waiter's
        # threshold can only be satisfied by that slot's own DMA chain, so a
        # fast SDMA engine running ahead on *other* DMAs can't mask a slow
        # engine's unfinished descriptors (cumulative counts on one shared sem
        # are unsound across concurrently in-flight DMAs)
        gsem = [gsem0, gsem1, gsem2, gsem3, gsem4, gsem5][:S]
        assert len(gsem) == S
        osem = [osem0, osem1]
        out_t = [out_t0, out_t1]

        def stg(s, j=0):
            # row j of staging slot s
            return stage[:, (s * PAIR + j) * D:(s * PAIR + j + 1) * D]

        @block.gpsimd
        def _(g):
            g.dma_start(out=idx_s[:], in_=idxs[:]).then_inc(load_sem, 16)
            g.dma_start(out=usth_s[:], in_=usth[:]).then_inc(load_sem, 16)
            g.dma_start(out=bias_s[:], in_=biasb[:]).then_inc(load_sem, 16)
            # descriptor generation reads idx_s from SBUF: loads must land first
            g.wait_ge(load_sem, LOADS)
            assert K % PAIR == 0  # a DMA's rows never straddle units
            for gp in range(NG // PAIR):
                u, k0 = (gp * PAIR) // K, (gp * PAIR) % K
                t, p = (u // 2) % NT, u % 2
                col = (2 * t + p) * K + k0
                if gp >= S:
                    # DVE must have consumed the staging slot's previous tenant
                    g.wait_ge(dve_sem, (gp - S + 1) * PAIR)
                g.indirect_dma_start(
                    out=stage[:, (gp % S) * PAIR * D:((gp % S) + 1) * PAIR * D],
                    out_offset=None,
                    in_=table[:],
                    in_offset=bass.IndirectOffsetOnAxis(
                        ap=idx_s[:, col:col + PAIR], axis=0
                    ),
                ).then_inc(gsem[gp % S], 16)

        @block.vector
        def _(v):
            v.wait_ge(load_sem, LOADS)   # usth_s/bias_s resident
            for it in range(repeat * NT):
                t, pb = it % NT, it % 2
                for p, acc in ((0, acc_w), (1, acc_b)):
                    u = 2 * it + p
                    for k in range(K):
                        gi = u * K + k
                        gp, j = gi // PAIR, gi % PAIR
                        if j == 0:
                            v.wait_ge(gsem[gp % S], 16 * (gp // S + 1))
                        src = bias_s if k == 0 else acc
                        # all-fp16 operands: DVE 2x_1p fast path
                        v.tensor_tensor(
                            out=acc[:], in0=src[:], in1=stg(gp % S, j),
                            op=mybir.AluOpType.add,
                        ).then_inc(dve_sem, 1)
                if it >= 2:
                    # SP must have drained out_t[pb] (tile it-2)
                    v.wait_ge(osem[pb], 16 * ((it - 2) // 2 + 1))
                w, b, o = acc_w, acc_b, out_t[pb]
                us = usth_s[:, t:t + 1]
                ush = usth_s[:, NT + t:NT + t + 1]        # us - 0.5
                nus = usth_s[:, 2 * NT + t:2 * NT + t + 1]  # -us
                # them == 1-us, so: first half = b + us*diff,
                # second half = w - us*diff, psqt = diff_psqt*(us-0.5)
                v.tensor_tensor(
                    out=diff[:], in0=w[:], in1=b[:],
                    op=mybir.AluOpType.subtract,
                )
                v.scalar_tensor_tensor(
                    o[:, 0:L1], diff[:, :L1], us, b[:, :L1],
                    op0=mybir.AluOpType.mult, op1=mybir.AluOpType.add,
                )
                v.tensor_scalar(
                    o[:, 0:L1], o[:, 0:L1], 0.0, ft_max,
                    op0=mybir.AluOpType.max, op1=mybir.AluOpType.min,
                )
                v.scalar_tensor_tensor(
                    o[:, L1:2 * L1], diff[:, :L1], nus, w[:, :L1],
                    op0=mybir.AluOpType.mult, op1=mybir.AluOpType.add,
                )
                v.tensor_scalar(
                    o[:, L1:2 * L1], o[:, L1:2 * L1], 0.0, ft_max,
                    op0=mybir.AluOpType.max, op1=mybir.AluOpType.min,
                )
                v.tensor_scalar_mul(
                    o[:, 2 * L1:OUTD], diff[:, L1:D], ush
                ).then_inc(combine_sem, 1)

        @block.sync
        def _(s):
            for it in range(repeat * NT):
                t, pb = it % NT, it % 2
                s.wait_ge(combine_sem, it + 1)
                s.dma_start(
                    out=out[t * 128:(t + 1) * 128, :], in_=out_t[pb][:]
                ).then_inc(osem[pb], 16)
            n = repeat * NT
            s.wait_ge(osem[0], 16 * ((n + 1) // 2))
            s.wait_ge(osem[1], 16 * (n // 2))

    return nc


def _prep_core_inputs(c, table, biasb, w_idx, b_idx, us, them):
    sl = slice(c * BPC, (c + 1) * BPC)
    wi = w_idx[sl].reshape(NT, 128, K)
    bi = b_idx[sl].reshape(NT, 128, K)
    blocks = []
    for t in range(NT):
        blocks.append(wi[t])
        blocks.append(bi[t])
    idxs = np.ascontiguousarray(np.concatenate(blocks, axis=1), dtype=np.int32)
    us_c = np.ascontiguousarray(us[sl, 0].reshape(NT, 128).T, dtype=np.float32)
    usth = np.concatenate([us_c, us_c - 0.5, -us_c], axis=1).astype(np.float32)
    return {"table": table, "biasb": biasb, "idxs": idxs, "usth": usth}


def run_on_hw(w_indices, w_values, b_indices, b_values, us, them, ft_max_val,
              merged_weight, bias, trace=False, repeat=1):
    """Run the device kernel; returns (output [B, OUTD], BassKernelResults)."""
    ft_max = float(np.asarray(ft_max_val))
    key = ("nc", ft_max, repeat)
    if key not in _CACHE:
        _CACHE[key] = build_nc(ft_max, repeat)
    nc = _CACHE[key]

    table = np.ascontiguousarray(
        np.asarray(merged_weight, dtype=np.float32).astype(np.float16)
    )
    biasb = np.ascontiguousarray(
        np.broadcast_to(np.asarray(bias, dtype=np.float32).astype(np.float16),
                        (128, D))
    )
    w_idx = np.asarray(w_indices, dtype=np.int64)
    b_idx = np.asarray(b_indices, dtype=np.int64)
    us = np.asarray(us, dtype=np.float32)
    them = np.asarray(them, dtype=np.float32)

    in_maps = [
        _prep_core_inputs(c, table, biasb, w_idx, b_idx, us, them)
        for c in range(NCORES)
    ]
    res = run_bass_kernel_spmd(nc, in_maps, list(range(NCORES)), trace=trace)
    outp = np.concatenate([res.results[c]["out"] for c in range(NCORES)], axis=0)
    return outp, res


def _host_fallback(w_indices, w_values, b_indices, b_values, us, them,
                   ft_max_val, merged_weight, bias):
    def acc(idx, val):
        rows = merged_weight[idx]
        return np.einsum('bk,bkd->bd', val, rows) + bias
    w = acc(w_indices, w_values)
    b = acc(b_indices, b_values)
    wacc, wpsqt = w[:, :L1], w[:, L1:]
    bacc, bpsqt = b[:, :L1], b[:, L1:]
    l0 = us * np.concatenate([wacc, bacc], axis=1) \
        + them * np.concatenate([bacc, wacc], axis=1)
    l0 = np.clip(l0, 0.0, np.float32(float(np.asarray(ft_max_val))))
    psqt = (wpsqt - bpsqt) * (us - 0.5)
    return np.concatenate([l0, psqt], axis=1).astype(np.float32)


def kernel(w_indices, w_values, b_indices, b_values, us, them, ft_max_val,
           merged_weight, bias):
    if not (np.all(np.asarray(w_values) == 1.0)
            and np.all(np.asarray(b_values) == 1.0)
            and np.array_equal(np.asarray(them, dtype=np.float32),
                               1.0 - np.asarray(us, dtype=np.float32))):
        # the device program folds the unit feature values into plain
        # accumulation and them==1-us into the combine; anything else is
        # out of spec — stay correct on host
        return _host_fallback(w_indices, w_values, b_indices, b_values, us,
                              them, ft_max_val, merged_weight, bias)
    outp, _ = run_on_hw(w_indices, w_values, b_indices, b_values, us, them,
                        ft_max_val, merged_weight, bias)
    return outp



# revision 26
# speedup vs baseline: 1.9865x; 1.5320x over previous
"""Trainium2 Bass kernel for nn_ComposedFeatureTransformer (NNUE-style double
feature transformer: sparse gather-accumulate + bias, perspective concat, clip,
psqt head).

Strategy: data-parallel over batch across 8 NeuronCores (512 items/core, table
replicated). Per core, items are processed in 4 tiles of 128 (one item per SBUF
partition); each tile needs two 32-row weighted sums (w/b perspectives).

Rows are fetched with indirect DMA (one [128, 3080] row-gather per feature
slot) into a ring of staging tiles; DVE accumulates staging tiles into the
per-perspective accumulator (first add seeds with the broadcast bias), then
combines perspectives (us/them mix + clip + psqt) into an output tile that SP
(HWDGE) streams to DRAM. All DMA writes are plain copies — no in-DMA
accumulate (the SDMA CCE read-modify-write is unsound for multi-packet rows).

The table, staging tiles, bias and accumulators are all fp16 (halves the
dominant gather traffic: 512*64 rows * 6160 B ~= 202 MB per core) so the
accumulate tensor_tensor adds take the DVE 2x_1p fast path (all-2-byte
operands, 2 col/cycle) and hide (~430 us) under the gather stream (~560 us).
Mixed-dtype adds would run 1x and become the bottleneck — measured 890 us
DVE-busy with a bf16-table/fp32-acc variant. fp16's 10 mantissa bits keep
the exact-emulation rel err at 2.2e-3 against the harness 2e-2 gate
(bf16 acc: 1.8e-2, fp8 table: 2.8e-2 — both rejected).

The combine uses them == 1-us (guaranteed by setup_inputs, checked at entry):
diff = w - b once per tile (fp16, 2x), then each output half is one
scalar_tensor_tensor against us / -us plus a clip; psqt reuses diff.

Requires w_values/b_values == 1 (guaranteed by the problem spec fill); falls
back to a host computation otherwise so kernel() stays correct on any input.
"""
import sys

if '/opt/trn_rl_repo' not in sys.path:
    sys.path.insert(0, '/opt/trn_rl_repo')

import numpy as np

import concourse.bass as bass
import concourse.mybir as mybir
from concourse.bass_utils import run_bass_kernel_spmd

L1 = 3072
NPSQT = 8
D = L1 + NPSQT            # 3080
V = 45056                 # table rows
K = 32                    # active features per perspective
B = 4096                  # batch
NCORES = 8
BPC = B // NCORES         # 512 items per core
NT = BPC // 128           # 4 item-tiles per core
OUTD = 2 * L1 + NPSQT     # 6152
# PAIR=2 (two rows per indirect DMA) was measured cost-model-neutral —
# descriptor generation is fully hidden behind the transfer stream — so the
# hardware-validated single-row configuration is kept.
PAIR = 1                  # table rows fetched per indirect DMA
S = 6                     # staging ring depth (slots of PAIR rows)

f32 = mybir.dt.float32
f16 = mybir.dt.float16
i32 = mybir.dt.int32

_CACHE = {}


def build_nc(ft_max: float, repeat: int = 1):
    """Build the single-core Bass program (shared SPMD across all 8 cores).

    repeat>1 re-runs the whole compute that many times (for timing the
    on-device portion by slope; the output is simply rewritten).
    """
    nc = bass.Bass()
    table = nc.declare_dram_parameter("table", [V, D], f16, isOutput=False)
    biasb = nc.declare_dram_parameter("biasb", [128, D], f16, isOutput=False)
    idxs = nc.declare_dram_parameter("idxs", [128, 2 * NT * K], i32, isOutput=False)
    usth = nc.declare_dram_parameter("usth", [128, 3 * NT], f32, isOutput=False)
    # fp16 output (host upconverts): halves write traffic and keeps the
    # combine all-2-byte for the DVE fast paths
    out = nc.declare_dram_parameter("out", [BPC, OUTD], f16, isOutput=True)

    NG = repeat * 2 * NT * K      # total gathers
    LOADS = 32                    # usth + bias preload sem ticks

    from contextlib import ExitStack

    with ExitStack() as _ctx:
        ec = _ctx.enter_context
        idx_s = ec(nc.sbuf_tensor([128, 2 * NT * K], i32))
        usth_s = ec(nc.sbuf_tensor([128, 3 * NT], f32))
        bias_s = ec(nc.sbuf_tensor([128, D], f16))
        acc_w = ec(nc.sbuf_tensor([128, D], f16))
        acc_b = ec(nc.sbuf_tensor([128, D], f16))
        stage = ec(nc.sbuf_tensor([128, S * PAIR * D], f16))
        out_t0 = ec(nc.sbuf_tensor([128, OUTD], f16))
        out_t1 = ec(nc.sbuf_tensor([128, OUTD], f16))
        diff = ec(nc.sbuf_tensor([128, D], f16))
        load_sem = ec(nc.semaphore("load_sem"))
        idx_sem = ec(nc.semaphore("idx_sem"))
        gsem0 = ec(nc.semaphore("gsem0"))
        gsem1 = ec(nc.semaphore("gsem1"))
        gsem2 = ec(nc.semaphore("gsem2"))
        gsem3 = ec(nc.semaphore("gsem3"))
        gsem4 = ec(nc.semaphore("gsem4"))
        gsem5 = ec(nc.semaphore("gsem5"))
        dve_sem = ec(nc.semaphore("dve_sem"))
        combine_sem = ec(nc.semaphore("combine_sem"))
        osem0 = ec(nc.semaphore("osem0"))
        osem1 = ec(nc.semaphore("osem1"))
        block = ec(nc.Block())
        # one completion sem per staging slot / output-tile parity: a waiter's
        # threshold can only be satisfied by that slot's own DMA chain, so a
        # fast SDMA engine running ahead on *other* DMAs can't mask a slow
        # engine's unfinished descriptors (cumulative counts on one shared sem
        # are unsound across concurrently in-flight DMAs)
        gsem = [gsem0, gsem1, gsem2, gsem3, gsem4, gsem5][:S]
        assert len(gsem) == S
        osem = [osem0, osem1]
        out_t = [out_t0, out_t1]

        def stg(s, j=0):
            # row j of staging slot s
            return stage[:, (s * PAIR + j) * D:(s * PAIR + j + 1) * D]

        @block.gpsimd
        def _(g):
            g.dma_start(out=idx_s[:], in_=idxs[:]).then_inc(idx_sem, 16)
            g.dma_start(out=usth_s[:], in_=usth[:]).then_inc(load_sem, 16)
            g.dma_start(out=bias_s[:], in_=biasb[:]).then_inc(load_sem, 16)
            # descriptor generation reads only idx_s; usth/bias gate DVE
            g.wait_ge(idx_sem, 16)
            assert K % PAIR == 0  # a DMA's rows never straddle units
            for gp in range(NG // PAIR):
                u, k0 = (gp * PAIR) // K, (gp * PAIR) % K
                t, p = (u // 2) % NT, u % 2
                col = (2 * t + p) * K + k0
                if gp >= S:
                    # DVE must have consumed the staging slot's previous tenant
                    g.wait_ge(dve_sem, (gp - S + 1) * PAIR)
                g.indirect_dma_start(
                    out=stage[:, (gp % S) * PAIR * D:((gp % S) + 1) * PAIR * D],
                    out_offset=None,
                    in_=table[:],
                    in_offset=bass.IndirectOffsetOnAxis(
                        ap=idx_s[:, col:col + PAIR], axis=0
                    ),
                ).then_inc(gsem[gp % S], 16)

        @block.vector
        def _(v):
            v.wait_ge(load_sem, LOADS)   # usth_s/bias_s resident
            for it in range(repeat * NT):
                t, pb = it % NT, it % 2
                for p, acc in ((0, acc_w), (1, acc_b)):
                    u = 2 * it + p
                    for k in range(K):
                        gi = u * K + k
                        gp, j = gi // PAIR, gi % PAIR
                        if j == 0:
                            v.wait_ge(gsem[gp % S], 16 * (gp // S + 1))
                        src = bias_s if k == 0 else acc
                        # all-fp16 operands: DVE 2x_1p fast path
                        v.tensor_tensor(
                            out=acc[:], in0=src[:], in1=stg(gp % S, j),
                            op=mybir.AluOpType.add,
                        ).then_inc(dve_sem, 1)
                if it >= 2:
                    # SP must have drained out_t[pb] (tile it-2, two writes)
                    v.wait_ge(osem[pb], 32 * ((it - 2) // 2 + 1))
                w, b, o = acc_w, acc_b, out_t[pb]
                us = usth_s[:, t:t + 1]
                ush = usth_s[:, NT + t:NT + t + 1]        # us - 0.5
                nus = usth_s[:, 2 * NT + t:2 * NT + t + 1]  # -us
                # them == 1-us, so: first half = b + us*diff,
                # second half = w - us*diff, psqt = diff_psqt*(us-0.5)
                v.tensor_tensor(
                    out=diff[:], in0=w[:], in1=b[:],
                    op=mybir.AluOpType.subtract,
                )
                v.scalar_tensor_tensor(
                    o[:, 0:L1], diff[:, :L1], us, b[:, :L1],
                    op0=mybir.AluOpType.mult, op1=mybir.AluOpType.add,
                )
                v.tensor_scalar(
                    o[:, 0:L1], o[:, 0:L1], 0.0, ft_max,
                    op0=mybir.AluOpType.max, op1=mybir.AluOpType.min,
                ).then_inc(combine_sem, 1)   # first half ready -> SP write 1
                v.scalar_tensor_tensor(
                    o[:, L1:2 * L1], diff[:, :L1], nus, w[:, :L1],
                    op0=mybir.AluOpType.mult, op1=mybir.AluOpType.add,
                )
                v.tensor_scalar(
                    o[:, L1:2 * L1], o[:, L1:2 * L1], 0.0, ft_max,
                    op0=mybir.AluOpType.max, op1=mybir.AluOpType.min,
                )
                v.tensor_scalar_mul(
                    o[:, 2 * L1:OUTD], diff[:, L1:D], ush
                ).then_inc(combine_sem, 1)

        @block.sync
        def _(s):
            # two half-writes per tile: the first L1 columns stream out while
            # DVE finishes the second half (shortens the end-of-kernel tail)
            for it in range(repeat * NT):
                t, pb = it % NT, it % 2
                s.wait_ge(combine_sem, 2 * it + 1)
                s.dma_start(
                    out=out[t * 128:(t + 1) * 128, :L1],
                    in_=out_t[pb][:, :L1],
                ).then_inc(osem[pb], 16)
                s.wait_ge(combine_sem, 2 * it + 2)
                s.dma_start(
                    out=out[t * 128:(t + 1) * 128, L1:],
                    in_=out_t[pb][:, L1:],
                ).then_inc(osem[pb], 16)
            n = repeat * NT
            s.wait_ge(osem[0], 32 * ((n + 1) // 2))
            s.wait_ge(osem[1], 32 * (n // 2))

    return nc


def _prep_core_inputs(c, table, biasb, w_idx, b_idx, us, them):
    sl = slice(c * BPC, (c + 1) * BPC)
    wi = w_idx[sl].reshape(NT, 128, K)
    bi = b_idx[sl].reshape(NT, 128, K)
    blocks = []
    for t in range(NT):
        blocks.append(wi[t])
        blocks.append(bi[t])
    idxs = np.ascontiguousarray(np.concatenate(blocks, axis=1), dtype=np.int32)
    us_c = np.ascontiguousarray(us[sl, 0].reshape(NT, 128).T, dtype=np.float32)
    usth = np.concatenate([us_c, us_c - 0.5, -us_c], axis=1).astype(np.float32)
    return {"table": table, "biasb": biasb, "idxs": idxs, "usth": usth}


def run_on_hw(w_indices, w_values, b_indices, b_values, us, them, ft_max_val,
              merged_weight, bias, trace=False, repeat=1):
    """Run the device kernel; returns (output [B, OUTD], BassKernelResults)."""
    ft_max = float(np.asarray(ft_max_val))
    key = ("nc", ft_max, repeat)
    if key not in _CACHE:
        _CACHE[key] = build_nc(ft_max, repeat)
    nc = _CACHE[key]

    table = np.ascontiguousarray(
        np.asarray(merged_weight, dtype=np.float32).astype(np.float16)
    )
    biasb = np.ascontiguousarray(
        np.broadcast_to(np.asarray(bias, dtype=np.float32).astype(np.float16),
                        (128, D))
    )
    w_idx = np.asarray(w_indices, dtype=np.int64)
    b_idx = np.asarray(b_indices, dtype=np.int64)
    us = np.asarray(us, dtype=np.float32)
    them = np.asarray(them, dtype=np.float32)

    in_maps = [
        _prep_core_inputs(c, table, biasb, w_idx, b_idx, us, them)
        for c in range(NCORES)
    ]
    res = run_bass_kernel_spmd(nc, in_maps, list(range(NCORES)), trace=trace)
    outp = np.concatenate(
        [np.asarray(res.results[c]["out"]) for c in range(NCORES)], axis=0
    ).astype(np.float32)
    return outp, res


def _host_fallback(w_indices, w_values, b_indices, b_values, us, them,
                   ft_max_val, merged_weight, bias):
    def acc(idx, val):
        rows = merged_weight[idx]
        return np.einsum('bk,bkd->bd', val, rows) + bias
    w = acc(w_indices, w_values)
    b = acc(b_indices, b_values)
    wacc, wpsqt = w[:, :L1], w[:, L1:]
    bacc, bpsqt = b[:, :L1], b[:, L1:]
    l0 = us * np.concatenate([wacc, bacc], axis=1) \
        + them * np.concatenate([bacc, wacc], axis=1)
    l0 = np.clip(l0, 0.0, np.float32(float(np.asarray(ft_max_val))))
    psqt = (wpsqt - bpsqt) * (us - 0.5)
    return np.concatenate([l0, psqt], axis=1).astype(np.float32)


def kernel(w_indices, w_values, b_indices, b_values, us, them, ft_max_val,
           merged_weight, bias):
    if not (np.all(np.asarray(w_values) == 1.0)
            and np.all(np.asarray(b_values) == 1.0)
            and np.array_equal(np.asarray(them, dtype=np.float32),
                               1.0 - np.asarray(us, dtype=np.float32))):
        # the device program folds the unit feature values into plain
        # accumulation and them==1-us into the combine; anything else is
        # out of spec — stay correct on host
        return _host_fallback(w_indices, w_values, b_indices, b_values, us,
                              them, ft_max_val, merged_weight, bias)
    outp, _ = run_on_hw(w_indices, w_values, b_indices, b_values, us, them,
                        ft_max_val, merged_weight, bias)
    return outp

